# revision 19
# baseline (speedup 1.0000x reference)
"""Banded DTW (window=100) on Trainium2, 8 NeuronCores.

Problem: x, y of shape (T=1024, N=32, C=4). Per trace n: banded DTW on the
(1024, 1024) pairwise-distance grid, band j in [i-100, i+100); cells outside
the band hold 0 (torch quirk); row 0 / col 0 seeded with raw distances.
Output: scalar mean over the 32 per-trace DTW values.

Strategy (data parallel over traces, 4 per core):
  Band-relative storage: row i keeps u in [0, 200], u = j - (i - 100).
  Row recurrence  cur[u] = min(min(prev[u], prev[u+1]), cur[u-1]) + d[u]
  maps to ONE hw scan:  tensor_tensor_scan(data0=m, data1=d, op0=min, op1=add)
  with m[u] = min(prev[u], prev[u+1]) (one tensor_tensor).  So 2 DVE ops/row.
  Out-of-band zeros, left-edge seeds and the sliding window are handled by
  poisoning the precomputed banded distance matrix (phase A) so the scan
  reproduces the reference semantics exactly (m[200] is kept 0; the poisoned
  d makes state reset to 0 across band edges).
"""

import os
import sys

import numpy as np

for _p in ("/opt/trn_rl_repo", "/root/.axon_site/_ro/trn_rl_repo"):
    if os.path.isdir(_p) and _p not in sys.path:
        sys.path.insert(0, _p)

import concourse.bass as bass
import concourse.bacc as bacc
import concourse.mybir as mybir
from concourse.bass_utils import run_bass_kernel_spmd
from concourse.tile import TileContext

T = 1024          # time steps (both sequences)
C = 4             # channels
N = 32            # traces
NCORES = 8
TPC = N // NCORES  # 4 traces per core
WIN = 100
BW = 2 * WIN + 1   # 201: band storage width, u in [0, 200]
YP = T + 2 * WIN   # 1224: padded y length
SLAB = 128         # phase-A rows per slab
CH = 64            # phase-B rows per streamed chunk

F32 = mybir.dt.float32
AF = mybir.ActivationFunctionType
OP = mybir.AluOpType

_CACHE = {}


def _build_nc():
    # Bacc (not raw Bass): its compile() pass splits multi-wait sync infos —
    # the TRN2 ISA allows at most one sync wait per instruction.
    nc = bacc.Bacc()
    x = nc.declare_dram_parameter("x", [TPC, T, C], F32, isOutput=False)
    ypad = nc.declare_dram_parameter("ypad", [TPC, C, YP], F32, isOutput=False)
    maskin = nc.declare_dram_parameter("maskin", [2, SLAB, BW], F32, isOutput=False)
    out = nc.declare_dram_parameter("out", [TPC, 1], F32, isOutput=True)

    with TileContext(nc) as tc:
        with (
            tc.tile_pool(name="const", bufs=1) as const,
            tc.tile_pool(name="pa", bufs=3) as pa,
            tc.tile_pool(name="dband", bufs=1, space="DRAM") as dram,
            tc.tile_pool(name="dchunk", bufs=2) as dchunk,
            tc.tile_pool(name="dp", bufs=1) as dp,
        ):
            dband = dram.tile([TPC * T * BW], F32, name="dband")

            mask0 = const.tile([SLAB, BW], F32)
            nc.sync.dma_start(mask0[:], maskin[0, :, :])
            maskr = const.tile([SLAB, BW], F32)
            nc.sync.dma_start(maskr[:], maskin[1, :, :])

            # ---------------- Phase A: banded distances -> DRAM -------------
            # D[i][u] = ||x[i] - y[i-100+u]||, i on partitions (slab of 128).
            # d^2 = sum_c y_c*(y_c - 2x_c) + sum_c x_c^2; the x-norm (plus a
            # tiny eps guarding fp32 cancellation) rides in the sqrt bias.
            # ACT instructions can carry only ONE sync wait, so everything
            # except the final biased sqrt stays on DVE.
            for t in range(TPC):
                for s in range(T // SLAB):
                    i0 = s * SLAB
                    xs = pa.tile([SLAB, C], F32, tag="xs")
                    nc.sync.dma_start(xs[:], x[t, i0 : i0 + SLAB, :])
                    x2 = pa.tile([SLAB, C], F32, tag="x2")
                    nc.vector.tensor_scalar(x2[:], xs[:], 2.0, None, OP.mult)
                    xsq = pa.tile([SLAB, C], F32, tag="xsq")
                    nc.vector.tensor_mul(xsq[:], xs[:], xs[:])
                    xn = pa.tile([SLAB, 1], F32, tag="xn")
                    nc.vector.tensor_reduce(
                        xn[:], xsq[:], axis=mybir.AxisListType.X, op=OP.add
                    )
                    xne = pa.tile([SLAB, 1], F32, tag="xne")
                    nc.vector.tensor_scalar(xne[:], xn[:], 1e-5, None, OP.add)

                    acc = pa.tile([SLAB, BW], F32, tag="acc")
                    for c in range(C):
                        ydc = pa.tile([SLAB, BW], F32, tag="ydc", bufs=8)
                        # ypad[t, c, i0 + p + u] : overlapping diagonal windows
                        src = bass.AP(
                            tensor=ypad,
                            offset=(t * C + c) * YP + i0,
                            ap=[[1, SLAB], [1, BW]],
                        )
                        nc.sync.dma_start(ydc[:], src)
                        if c == 0:
                            nc.vector.scalar_tensor_tensor(
                                acc[:], ydc[:], x2[:, 0:1], ydc[:],
                                op0=OP.subtract, op1=OP.mult,
                            )
                        else:
                            sq = pa.tile([SLAB, BW], F32, tag="sq", bufs=4)
                            nc.vector.scalar_tensor_tensor(
                                sq[:], ydc[:], x2[:, c : c + 1], ydc[:],
                                op0=OP.subtract, op1=OP.mult,
                            )
                            nc.vector.tensor_add(acc[:], acc[:], sq[:])
                    # ACT can encode only one sync wait: write a fresh tile
                    # whose only readers are DVE, so the sqrt waits on DVE
                    # alone; the mask-mul output (dmm) is what DMA reads.
                    dout = pa.tile([SLAB, BW], F32, tag="dout")
                    nc.scalar.activation(dout[:], acc[:], AF.Sqrt, bias=xne[:, 0:1])
                    # slab 0: zero the virtual (j<0) triangle and col 200 for
                    # rows>=1 (row 0 keeps its seeded d[0][100] at u=200).
                    # other slabs: zero col 200 everywhere.
                    dmm = pa.tile([SLAB, BW], F32, tag="dmm")
                    nc.vector.tensor_mul(
                        dmm[:], dout[:], mask0[:] if s == 0 else maskr[:]
                    )
                    dst = bass.AP(
                        tensor=dband.tensor,
                        offset=dband.offset + (t * T + i0) * BW,
                        ap=[[BW, SLAB], [1, BW]],
                    )
                    nc.sync.dma_start(dst, dmm[:])

            # phase boundary: joins all DMA-lane clocks so phase-B DMAs don't
            # accumulate per-lane RAW waits against phase-A dband writes.
            tc.strict_bb_all_engine_barrier()

            # ---------------- seeds: d[i][0] needed for row 101 initial -----
            x101 = dp.tile([TPC, C], F32)
            nc.sync.dma_start(x101[:], x[:, 101, :])
            y0 = dp.tile([TPC, C], F32)
            nc.sync.dma_start(
                y0[:],
                bass.AP(tensor=ypad, offset=WIN, ap=[[C * YP, TPC], [YP, C]]),
            )
            sdif = dp.tile([TPC, C], F32)
            nc.vector.tensor_sub(sdif[:], x101[:], y0[:])
            nc.vector.tensor_mul(sdif[:], sdif[:], sdif[:])
            seed = dp.tile([TPC, 1], F32)
            nc.vector.tensor_reduce(seed[:], sdif[:], axis=mybir.AxisListType.X, op=OP.add)
            nc.scalar.activation(seed[:], seed[:], AF.Sqrt)

            # ---------------- Phase B: the serial DP ------------------------
            prev = dp.tile([TPC, BW], F32)
            cur = dp.tile([TPC, BW], F32)
            m = dp.tile([TPC, BW], F32)
            nc.gpsimd.memset(m[:], 0.0)  # m[200] stays 0 forever

            nc.sync.dma_start(
                prev[0:TPC, :],
                bass.AP(
                    tensor=dband.tensor,
                    offset=dband.offset,
                    ap=[[T * BW, TPC], [1, BW]],
                ),
            )

            for ch in range(T // CH):
                cht = dchunk.tile([TPC, CH * BW], F32, tag="chunk")
                nc.sync.dma_start(
                    cht[0:TPC, :],
                    bass.AP(
                        tensor=dband.tensor,
                        offset=dband.offset + ch * CH * BW,
                        ap=[[T * BW, TPC], [1, CH * BW]],
                    ),
                )
                for li in range(CH):
                    i = ch * CH + li
                    if i == 0:
                        continue
                    drow = cht[0:TPC, li * BW : (li + 1) * BW]
                    nc.vector.tensor_tensor(
                        m[0:TPC, 0 : BW - 1],
                        prev[0:TPC, 0 : BW - 1],
                        prev[0:TPC, 1:BW],
                        OP.min,
                    )
                    nc.vector.tensor_tensor_scan(
                        cur[0:TPC, :],
                        m[0:TPC, :],
                        drow,
                        seed[0:TPC, 0:1] if i == WIN + 1 else 0.0,
                        op0=OP.min,
                        op1=OP.add,
                    )
                    prev, cur = cur, prev

            nc.sync.dma_start(out[:, :], prev[0:TPC, WIN : WIN + 1])
    if not nc.is_finalized():
        nc.finalize()  # runs Bacc.compile(): wait-splitting + reg alloc
    return nc


def _host_mask():
    p = np.arange(SLAB)[:, None]
    u = np.arange(BW)[None, :]
    mask0 = ((u + p) > 99.5).astype(np.float32)
    mask0[1:, BW - 1] = 0.0
    maskr = np.ones((SLAB, BW), dtype=np.float32)
    maskr[:, BW - 1] = 0.0
    return np.stack([mask0, maskr])


def _shard_inputs(x, y):
    """x, y: (T, N, C) full -> per-core input maps."""
    xt = np.ascontiguousarray(x.transpose(1, 0, 2)).astype(np.float32)  # (N,T,C)
    yt = y.transpose(1, 0, 2).astype(np.float32)
    ypad = np.zeros((N, C, YP), dtype=np.float32)
    ypad[:, :, WIN : WIN + T] = yt.transpose(0, 2, 1)
    mask = _host_mask()
    in_maps = []
    for k in range(NCORES):
        sl = slice(k * TPC, (k + 1) * TPC)
        in_maps.append(
            {
                "x": np.ascontiguousarray(xt[sl]),
                "ypad": np.ascontiguousarray(ypad[sl]),
                "maskin": mask,
            }
        )
    return in_maps


LAST_RESULTS = None


def kernel(x, y, _trace=False):
    global LAST_RESULTS
    if "nc" not in _CACHE:
        _CACHE["nc"] = _build_nc()
    nc = _CACHE["nc"]
    in_maps = _shard_inputs(np.asarray(x), np.asarray(y))
    res = run_bass_kernel_spmd(
        nc, in_maps, list(range(NCORES)), trace=_trace
    )
    LAST_RESULTS = res
    vals = np.concatenate([r["out"].reshape(-1) for r in res.results])
    return np.float32(vals.astype(np.float32).sum() / np.float32(N))


# revision 29
# speedup vs baseline: 1.0286x; 1.0286x over previous
"""Banded DTW (window=100) on Trainium2, 8 NeuronCores.

Problem: x, y of shape (T=1024, N=32, C=4). Per trace n: banded DTW on the
(1024, 1024) pairwise-distance grid, band j in [i-100, i+100); cells outside
the band hold 0 (torch quirk); row 0 / col 0 seeded with raw distances.
Output: scalar mean over the 32 per-trace DTW values.

Strategy (data parallel over traces, 4 per core):
  Band-relative storage: row i keeps u in [0, 200], u = j - (i - 100).
  Row recurrence  cur[u] = min(min(prev[u], prev[u+1]), cur[u-1]) + d[u]
  maps to ONE hw scan:  tensor_tensor_scan(data0=m, data1=d, op0=min, op1=add)
  with m[u] = min(prev[u], prev[u+1]) (one tensor_tensor).  So 2 DVE ops/row.
  Out-of-band zeros, left-edge seeds and the sliding window are handled by
  poisoning the precomputed banded distance matrix (phase A) so the scan
  reproduces the reference semantics exactly (m[200] is kept 0; the poisoned
  d makes state reset to 0 across band edges).
"""

import os
import sys

import numpy as np

for _p in ("/opt/trn_rl_repo", "/root/.axon_site/_ro/trn_rl_repo"):
    if os.path.isdir(_p) and _p not in sys.path:
        sys.path.insert(0, _p)

import concourse.bass as bass
import concourse.bacc as bacc
import concourse.mybir as mybir
from concourse.bass_utils import run_bass_kernel_spmd
from concourse.tile import TileContext

T = 1024          # time steps (both sequences)
C = 4             # channels
N = 32            # traces
NCORES = 8
TPC = N // NCORES  # 4 traces per core
WIN = 100
BW = 2 * WIN + 1   # 201: band storage width, u in [0, 200]
YP = T + 2 * WIN   # 1224: padded y length
SLAB = 128         # phase-A rows per slab
CH = 64            # phase-B rows per streamed chunk

F32 = mybir.dt.float32
AF = mybir.ActivationFunctionType
OP = mybir.AluOpType

_CACHE = {}


def _build_nc():
    # Bacc (not raw Bass): its compile() pass splits multi-wait sync infos —
    # the TRN2 ISA allows at most one sync wait per instruction.
    nc = bacc.Bacc()
    x = nc.declare_dram_parameter("x", [TPC, T, C], F32, isOutput=False)
    ypad = nc.declare_dram_parameter("ypad", [TPC, C, YP], F32, isOutput=False)
    maskin = nc.declare_dram_parameter("maskin", [2, SLAB, BW], F32, isOutput=False)
    out = nc.declare_dram_parameter("out", [TPC, 1], F32, isOutput=True)

    with TileContext(nc) as tc:
        with (
            tc.tile_pool(name="const", bufs=1) as const,
            tc.tile_pool(name="pa", bufs=3) as pa,
            tc.tile_pool(name="dband", bufs=1, space="DRAM") as dram,
            tc.tile_pool(name="dchunk", bufs=2) as dchunk,
            tc.tile_pool(name="dp", bufs=1) as dp,
        ):
            # one DRAM tile per 64-row chunk so phase-B reads depend only on
            # the phase-A slabs that produced that chunk (A/B overlap).
            dband = [
                dram.tile([TPC, CH * BW], F32, tag=f"dbc{c}", name=f"dband{c}")
                for c in range(T // CH)
            ]

            mask0 = const.tile([SLAB, BW], F32)
            nc.sync.dma_start(mask0[:], maskin[0, :, :])
            maskr = const.tile([SLAB, BW], F32)
            nc.sync.dma_start(maskr[:], maskin[1, :, :])

            # ---------------- seeds: d[i][0] needed for row 101 initial -----
            x101 = dp.tile([TPC, C], F32)
            nc.sync.dma_start(x101[:], x[:, 101, :])
            y0 = dp.tile([TPC, C], F32)
            nc.sync.dma_start(
                y0[:],
                bass.AP(tensor=ypad, offset=WIN, ap=[[C * YP, TPC], [YP, C]]),
            )
            sdif = dp.tile([TPC, C], F32)
            nc.vector.tensor_sub(sdif[:], x101[:], y0[:])
            nc.vector.tensor_mul(sdif[:], sdif[:], sdif[:])
            seed = dp.tile([TPC, 1], F32)
            nc.vector.tensor_reduce(
                seed[:], sdif[:], axis=mybir.AxisListType.X, op=OP.add
            )
            nc.scalar.activation(seed[:], seed[:], AF.Sqrt)

            # ---------------- Phase A: banded distances -> DRAM -------------
            # D[i][u] = ||x[i] - y[i-100+u]||, i on partitions (slab of 128).
            # sq_c = (y_c - x_c)^2 via ACT Square with per-partition bias
            # (exact, no cancellation); adds + mask on GPSIMD; DVE stays free
            # for the phase-B DP chain. Slab loop is s-outer so chunks
            # complete in the order phase B consumes them.
            for s in range(T // SLAB):
                i0 = s * SLAB
                for t in range(TPC):
                    xs = pa.tile([SLAB, C], F32, tag="xs")
                    nc.sync.dma_start(xs[:], x[t, i0 : i0 + SLAB, :])
                    xneg = pa.tile([SLAB, C], F32, tag="xneg")
                    nc.scalar.mul(xneg[:], xs[:], -1.0)

                    acc = pa.tile([SLAB, BW], F32, tag="acc")
                    for c in range(C):
                        ydc = pa.tile([SLAB, BW], F32, tag="ydc", bufs=8)
                        # ypad[t, c, i0 + p + u] : overlapping diagonal windows
                        src = bass.AP(
                            tensor=ypad,
                            offset=(t * C + c) * YP + i0,
                            ap=[[1, SLAB], [1, BW]],
                        )
                        nc.sync.dma_start(ydc[:], src)
                        if c == 0:
                            nc.scalar.activation(
                                acc[:], ydc[:], AF.Square, bias=xneg[:, 0:1]
                            )
                        else:
                            sq = pa.tile([SLAB, BW], F32, tag="sq", bufs=4)
                            nc.scalar.activation(
                                sq[:], ydc[:], AF.Square, bias=xneg[:, c : c + 1]
                            )
                            nc.gpsimd.tensor_add(acc[:], acc[:], sq[:])
                    dout = pa.tile([SLAB, BW], F32, tag="dout")
                    nc.scalar.activation(dout[:], acc[:], AF.Sqrt)
                    # slab 0: zero the virtual (j<0) triangle and col 200 for
                    # rows>=1 (row 0 keeps its seeded d[0][100] at u=200).
                    # other slabs: zero col 200 everywhere.
                    dmm = pa.tile([SLAB, BW], F32, tag="dmm")
                    nc.gpsimd.tensor_mul(
                        dmm[:], dout[:], mask0[:] if s == 0 else maskr[:]
                    )
                    # slab rows [i0, i0+128) span chunk tiles 2s and 2s+1
                    for half in range(2):
                        cti = 2 * s + half
                        dst = bass.AP(
                            tensor=dband[cti].tensor,
                            offset=dband[cti].offset + t * CH * BW,
                            ap=[[BW, CH], [1, BW]],
                        )
                        nc.sync.dma_start(
                            dst, dmm[half * CH : (half + 1) * CH, :]
                        )

            # ---------------- Phase B: the serial DP ------------------------
            prev = dp.tile([TPC, BW], F32)
            cur = dp.tile([TPC, BW], F32)
            m = dp.tile([TPC, BW], F32)
            nc.gpsimd.memset(m[:], 0.0)  # m[200] stays 0 forever
            # zero-init both DP buffers: the virtual (j<0) prefix of each row
            # is never written by the trimmed scans and must read as 0.
            nc.gpsimd.memset(prev[:], 0.0)
            nc.gpsimd.memset(cur[:], 0.0)

            nc.sync.dma_start(prev[0:TPC, :], dband[0][0:TPC, 0:BW])

            for ch in range(T // CH):
                cht = dchunk.tile([TPC, CH * BW], F32, tag="chunk")
                nc.sync.dma_start(cht[0:TPC, :], dband[ch][0:TPC, :])
                for li in range(CH):
                    i = ch * CH + li
                    if i == 0:
                        continue
                    # real band cells: u in [us, ue); outside is either the
                    # virtual j<0 region (top rows; state stays 0 past it so
                    # skipping is exact) or j>1023 garbage (bottom rows;
                    # never read by later real cells).
                    us = max(0, WIN - i)
                    ue = min(BW, T + WIN - i)  # covers last real u (1123-i)
                    drow = cht[0:TPC, li * BW + us : li * BW + ue]
                    # full rows: m[200] is the preset 0 (prev[201] doesn't
                    # exist); trimmed bottom rows: the last real cell (j=1023)
                    # needs m[ue-1] = min(prev[ue-1], prev[ue]) computed.
                    me = ue - 1 if ue == BW else ue
                    nc.vector.tensor_tensor(
                        m[0:TPC, us:me],
                        prev[0:TPC, us:me],
                        prev[0:TPC, us + 1 : me + 1],
                        OP.min,
                    )
                    nc.vector.tensor_tensor_scan(
                        cur[0:TPC, us:ue],
                        m[0:TPC, us:ue],
                        drow,
                        seed[0:TPC, 0:1] if i == WIN + 1 else 0.0,
                        op0=OP.min,
                        op1=OP.add,
                    )
                    prev, cur = cur, prev

            nc.sync.dma_start(out[:, :], prev[0:TPC, WIN : WIN + 1])
    if not nc.is_finalized():
        nc.finalize()  # runs Bacc.compile(): wait-splitting + reg alloc
    return nc


def _host_mask():
    p = np.arange(SLAB)[:, None]
    u = np.arange(BW)[None, :]
    mask0 = ((u + p) > 99.5).astype(np.float32)
    mask0[1:, BW - 1] = 0.0
    maskr = np.ones((SLAB, BW), dtype=np.float32)
    maskr[:, BW - 1] = 0.0
    return np.stack([mask0, maskr])


def _shard_inputs(x, y):
    """x, y: (T, N, C) full -> per-core input maps."""
    xt = np.ascontiguousarray(x.transpose(1, 0, 2)).astype(np.float32)  # (N,T,C)
    yt = y.transpose(1, 0, 2).astype(np.float32)
    ypad = np.zeros((N, C, YP), dtype=np.float32)
    ypad[:, :, WIN : WIN + T] = yt.transpose(0, 2, 1)
    mask = _host_mask()
    in_maps = []
    for k in range(NCORES):
        sl = slice(k * TPC, (k + 1) * TPC)
        in_maps.append(
            {
                "x": np.ascontiguousarray(xt[sl]),
                "ypad": np.ascontiguousarray(ypad[sl]),
                "maskin": mask,
            }
        )
    return in_maps


LAST_RESULTS = None


def kernel(x, y, _trace=False):
    global LAST_RESULTS
    if "nc" not in _CACHE:
        _CACHE["nc"] = _build_nc()
    nc = _CACHE["nc"]
    in_maps = _shard_inputs(np.asarray(x), np.asarray(y))
    res = run_bass_kernel_spmd(
        nc, in_maps, list(range(NCORES)), trace=_trace
    )
    LAST_RESULTS = res
    vals = np.concatenate([r["out"].reshape(-1) for r in res.results])
    return np.float32(vals.astype(np.float32).sum() / np.float32(N))


# revision 34
# speedup vs baseline: 1.1346x; 1.1030x over previous
"""Banded DTW (window=100) on Trainium2, 8 NeuronCores.

Problem: x, y of shape (T=1024, N=32, C=4). Per trace n: banded DTW on the
(1024, 1024) pairwise-distance grid, band j in [i-100, i+100); cells outside
the band hold 0 (torch quirk); row 0 / col 0 seeded with raw distances.
Output: scalar mean over the 32 per-trace DTW values.

Strategy (data parallel over traces, 4 per core):
  Band-relative storage: row i keeps u in [0, 200], u = j - (i - 100).
  Row recurrence  cur[u] = min(min(prev[u], prev[u+1]), cur[u-1]) + d[u]
  maps to ONE hw scan:  tensor_tensor_scan(data0=m, data1=d, op0=min, op1=add)
  with m[u] = min(prev[u], prev[u+1]) (one tensor_tensor).  So 2 DVE ops/row.
  Out-of-band zeros, left-edge seeds and the sliding window are handled by
  poisoning the precomputed banded distance matrix (phase A) so the scan
  reproduces the reference semantics exactly (m[200] is kept 0; the poisoned
  d makes state reset to 0 across band edges).
"""

import os
import sys

import numpy as np

for _p in ("/opt/trn_rl_repo", "/root/.axon_site/_ro/trn_rl_repo"):
    if os.path.isdir(_p) and _p not in sys.path:
        sys.path.insert(0, _p)

import concourse.bass as bass
import concourse.bacc as bacc
import concourse.mybir as mybir
from concourse.bass_utils import run_bass_kernel_spmd
from concourse.tile import TileContext

T = 1024          # time steps (both sequences)
C = 4             # channels
N = 32            # traces
NCORES = 8
TPC = N // NCORES  # 4 traces per core
WIN = 100
BW = 2 * WIN + 1   # 201: band storage width, u in [0, 200]
YP = T + 2 * WIN   # 1224: padded y length
SLAB = 128         # phase-A rows per slab
CH = 64            # phase-B rows per streamed chunk

F32 = mybir.dt.float32
AF = mybir.ActivationFunctionType
OP = mybir.AluOpType

_CACHE = {}


def _build_nc():
    # Bacc (not raw Bass): its compile() pass splits multi-wait sync infos —
    # the TRN2 ISA allows at most one sync wait per instruction.
    nc = bacc.Bacc()
    x = nc.declare_dram_parameter("x", [TPC, T, C], F32, isOutput=False)
    ypad = nc.declare_dram_parameter("ypad", [TPC, C, YP], F32, isOutput=False)
    maskin = nc.declare_dram_parameter("maskin", [2, SLAB, BW], F32, isOutput=False)
    out = nc.declare_dram_parameter("out", [TPC, 1], F32, isOutput=True)

    with TileContext(nc) as tc:
        with (
            tc.tile_pool(name="const", bufs=1) as const,
            tc.tile_pool(name="pa", bufs=3) as pa,
            tc.tile_pool(name="dband", bufs=1, space="DRAM") as dram,
            tc.tile_pool(name="dchunk", bufs=2) as dchunk,
            tc.tile_pool(name="dp", bufs=1) as dp,
        ):
            # one DRAM tile per 128-row slab so phase-B reads depend only on
            # the phase-A slabs that produced that chunk (A/B overlap).
            dband = [
                dram.tile([TPC, SLAB * BW], F32, tag=f"dbs{s}", name=f"dband{s}")
                for s in range(T // SLAB)
            ]

            mask0 = const.tile([SLAB, BW], F32)
            nc.sync.dma_start(mask0[:], maskin[0, :, :])
            maskr = const.tile([SLAB, BW], F32)
            nc.sync.dma_start(maskr[:], maskin[1, :, :])

            # ---------------- seeds: d[i][0] needed for row 101 initial -----
            x101 = dp.tile([TPC, C], F32)
            nc.sync.dma_start(x101[:], x[:, 101, :])
            y0 = dp.tile([TPC, C], F32)
            nc.sync.dma_start(
                y0[:],
                bass.AP(tensor=ypad, offset=WIN, ap=[[C * YP, TPC], [YP, C]]),
            )
            sdif = dp.tile([TPC, C], F32)
            nc.vector.tensor_sub(sdif[:], x101[:], y0[:])
            nc.vector.tensor_mul(sdif[:], sdif[:], sdif[:])
            seed = dp.tile([TPC, 1], F32)
            nc.vector.tensor_reduce(
                seed[:], sdif[:], axis=mybir.AxisListType.X, op=OP.add
            )
            nc.scalar.activation(seed[:], seed[:], AF.Sqrt)

            # DP-state tiles + memsets, emitted BEFORE phase A so the Pool
            # queue clears them immediately and the DVE chain can start as
            # soon as the first chunk lands.
            prev = dp.tile([TPC, BW], F32)
            cur = dp.tile([TPC, BW], F32)
            m = dp.tile([TPC, BW], F32)
            nc.gpsimd.memset(m[:], 0.0)  # m[200] stays 0 forever
            # zero-init both DP buffers: the virtual (j<0) prefix of each row
            # is never written by the trimmed scans and must read as 0.
            nc.gpsimd.memset(prev[:], 0.0)
            nc.gpsimd.memset(cur[:], 0.0)

            # ---------------- Phase A: banded distances -> DRAM -------------
            # D[i][u] = ||x[i] - y[i-100+u]||, i on partitions (slab of 128).
            # sq_c = (y_c - x_c)^2 via ACT Square with per-partition bias
            # (exact, no cancellation); adds + mask on GPSIMD; DVE stays free
            # for the phase-B DP chain. Slab loop is s-outer so chunks
            # complete in the order phase B consumes them.
            for s in range(T // SLAB):
                i0 = s * SLAB
                for t in range(TPC):
                    # phase-A DMAs ride the ACT HWDGE ring (nc.scalar), not
                    # SP: the SP sequencer issues in order, and ~600ns per
                    # DMA issue would stall phase-B's chunk DMAs behind all
                    # of phase A (measured 163us of DVE idle).
                    xs = pa.tile([SLAB, C], F32, tag="xs")
                    nc.scalar.dma_start(xs[:], x[t, i0 : i0 + SLAB, :])
                    xneg = pa.tile([SLAB, C], F32, tag="xneg")
                    nc.scalar.mul(xneg[:], xs[:], -1.0)

                    # all 4 channels in one DMA: ydall[p, c*BW+u] =
                    # ypad[t, c, i0 + p + u] (overlapping diagonal windows)
                    ydall = pa.tile([SLAB, C * BW], F32, tag="ydall", bufs=3)
                    src = bass.AP(
                        tensor=ypad,
                        offset=t * C * YP + i0,
                        ap=[[1, SLAB], [YP, C], [1, BW]],
                    )
                    nc.scalar.dma_start(ydall[:], src)
                    acc = pa.tile([SLAB, BW], F32, tag="acc")
                    for c in range(C):
                        ydc = ydall[:, c * BW : (c + 1) * BW]
                        if c == 0:
                            nc.scalar.activation(
                                acc[:], ydc, AF.Square, bias=xneg[:, 0:1]
                            )
                        else:
                            sq = pa.tile([SLAB, BW], F32, tag="sq", bufs=4)
                            nc.scalar.activation(
                                sq[:], ydc, AF.Square, bias=xneg[:, c : c + 1]
                            )
                            nc.gpsimd.tensor_add(acc[:], acc[:], sq[:])
                    dout = pa.tile([SLAB, BW], F32, tag="dout")
                    nc.scalar.activation(dout[:], acc[:], AF.Sqrt)
                    # slab 0: zero the virtual (j<0) triangle and col 200 for
                    # rows>=1 (row 0 keeps its seeded d[0][100] at u=200).
                    # other slabs: zero col 200 everywhere.
                    dmm = pa.tile([SLAB, BW], F32, tag="dmm")
                    nc.gpsimd.tensor_mul(
                        dmm[:], dout[:], mask0[:] if s == 0 else maskr[:]
                    )
                    dst = bass.AP(
                        tensor=dband[s].tensor,
                        offset=dband[s].offset + t * SLAB * BW,
                        ap=[[BW, SLAB], [1, BW]],
                    )
                    nc.scalar.dma_start(dst, dmm[:])

            # ---------------- Phase B: the serial DP ------------------------
            nc.sync.dma_start(prev[0:TPC, :], dband[0][0:TPC, 0:BW])

            for ch in range(T // CH):
                cht = dchunk.tile([TPC, CH * BW], F32, tag="chunk")
                nc.sync.dma_start(
                    cht[0:TPC, :],
                    dband[ch // 2][0:TPC, (ch % 2) * CH * BW : (ch % 2 + 1) * CH * BW],
                )
                for li in range(CH):
                    i = ch * CH + li
                    if i == 0:
                        continue
                    # real band cells: u in [us, ue); outside is either the
                    # virtual j<0 region (top rows; state stays 0 past it so
                    # skipping is exact) or j>1023 garbage (bottom rows;
                    # never read by later real cells).
                    us = max(0, WIN - i)
                    ue = min(BW, T + WIN - i)  # covers last real u (1123-i)
                    drow = cht[0:TPC, li * BW + us : li * BW + ue]
                    # full rows: m[200] is the preset 0 (prev[201] doesn't
                    # exist); trimmed bottom rows: the last real cell (j=1023)
                    # needs m[ue-1] = min(prev[ue-1], prev[ue]) computed.
                    me = ue - 1 if ue == BW else ue
                    nc.vector.tensor_tensor(
                        m[0:TPC, us:me],
                        prev[0:TPC, us:me],
                        prev[0:TPC, us + 1 : me + 1],
                        OP.min,
                    )
                    nc.vector.tensor_tensor_scan(
                        cur[0:TPC, us:ue],
                        m[0:TPC, us:ue],
                        drow,
                        seed[0:TPC, 0:1] if i == WIN + 1 else 0.0,
                        op0=OP.min,
                        op1=OP.add,
                    )
                    prev, cur = cur, prev

            nc.sync.dma_start(out[:, :], prev[0:TPC, WIN : WIN + 1])
    if not nc.is_finalized():
        nc.finalize()  # runs Bacc.compile(): wait-splitting + reg alloc
    return nc


def _host_mask():
    p = np.arange(SLAB)[:, None]
    u = np.arange(BW)[None, :]
    mask0 = ((u + p) > 99.5).astype(np.float32)
    mask0[1:, BW - 1] = 0.0
    maskr = np.ones((SLAB, BW), dtype=np.float32)
    maskr[:, BW - 1] = 0.0
    return np.stack([mask0, maskr])


def _shard_inputs(x, y):
    """x, y: (T, N, C) full -> per-core input maps."""
    xt = np.ascontiguousarray(x.transpose(1, 0, 2)).astype(np.float32)  # (N,T,C)
    yt = y.transpose(1, 0, 2).astype(np.float32)
    ypad = np.zeros((N, C, YP), dtype=np.float32)
    ypad[:, :, WIN : WIN + T] = yt.transpose(0, 2, 1)
    mask = _host_mask()
    in_maps = []
    for k in range(NCORES):
        sl = slice(k * TPC, (k + 1) * TPC)
        in_maps.append(
            {
                "x": np.ascontiguousarray(xt[sl]),
                "ypad": np.ascontiguousarray(ypad[sl]),
                "maskin": mask,
            }
        )
    return in_maps


LAST_RESULTS = None


def kernel(x, y, _trace=False):
    global LAST_RESULTS
    if "nc" not in _CACHE:
        _CACHE["nc"] = _build_nc()
    nc = _CACHE["nc"]
    in_maps = _shard_inputs(np.asarray(x), np.asarray(y))
    res = run_bass_kernel_spmd(
        nc, in_maps, list(range(NCORES)), trace=_trace
    )
    LAST_RESULTS = res
    vals = np.concatenate([r["out"].reshape(-1) for r in res.results])
    return np.float32(vals.astype(np.float32).sum() / np.float32(N))
